# revision 13
# baseline (speedup 1.0000x reference)
"""MixGARCH Trainium2 kernel (pair-decimated scan, parity-split input).

Reference: scan over t of v_t = relu(bias + Wx @ o_t^2 + Wh * v_{t-1}) + 1e-6.
All quantities are >= 0 so relu is identity -> LINEAR diagonal recurrence
    v_t = Wh * v_{t-1} + c_t,    c_t = (bias + 1e-6) + Wx @ o_t^2

Per core (8 cores, full I/O): 65536 steps as 2 halves (lanes 64h+k).
Non-initial halves get a 1024-step warmup; core0/half0 uses the exact v0.

Pair decimation: with w_m = v_{2m},
    w_m = Wh^2 w_{m-1} + u_m,     u_m = c_{2m} + Wh c_{2m-1}
    v_{2m+1} = Wh w_m + c_{2m+1}
The DVE scan runs at HALF resolution (u only). Odd steps are reconstructed
and the work is split across engines to balance the pipeline: some tiles via
DVE scalar_tensor_tensor straight from PSUM, the rest via ACT evacuate + ACT
per-partition scale + GPSIMD add.

Input is packed parity-separated: even timesteps in one 32-row block, odd
(shifted by 1) in another, per half -> u is ONE 64-contract matmul per half
(no PSUM accumulation groups, which break codegen here), all rhs contiguous.
Chunks (3 per half) overlay on distinct partition row groups; per-chunk
weight variants select the right rows. Bias rides a ones-channel row.

The linear system is scaled x1024 so outputs fit fp16 normal range: output
DMA is half the bytes. Host interleaves parities and divides by 1024.
"""

import os
import numpy as np

T = 524288
K = 64
NJ = 8
NCORES = 8
W = 1024                # warmup steps per half
HALF = 32768            # real steps per half
TT = W + HALF           # 33792 steps per half-stream
NCH = 3                 # chunks per half
CHUNK = TT // NCH       # 11264 timesteps per chunk
PT = 512                # pairs per tile
PAIRS = TT // 2         # 16896 pairs per half
PPC = CHUNK // 2        # 5632 pairs per chunk
SCOL = PPC + 1          # stored cols per parity row (odd needs +1 lead col)
TPC = PPC // PT         # 11 tiles per chunk
NTILES = NCH * TPC      # 33
SCALE = 1024.0
NLOAD = 4               # xin DMA pieces
DVE_ODD_MOD = int(os.environ.get("MIXGARCH_DVE_ODD_MOD", "4"))

_CACHE = {}


def _build_nc():
    import concourse.bacc as bacc
    import concourse.mybir as mybir
    import concourse.tile as tile

    f32 = mybir.dt.float32
    bf16 = mybir.dt.bfloat16
    fp16 = mybir.dt.float16

    nc = bacc.Bacc(None, target_bir_lowering=False)
    xin = nc.dram_tensor("xin", [128, SCOL], f32, kind="ExternalInput")
    wt = nc.dram_tensor("wt", [128, NCH * 256], f32, kind="ExternalInput")
    whv = nc.dram_tensor("whv", [128, 1], f32, kind="ExternalInput")
    ws2 = nc.dram_tensor("ws2", [128, PT], f32, kind="ExternalInput")
    vinit = nc.dram_tensor("vinit", [128, 1], f32, kind="ExternalInput")
    voutE = nc.dram_tensor("voutE", [128, PAIRS], fp16, kind="ExternalOutput")
    voutO = nc.dram_tensor("voutO", [128, PAIRS], fp16, kind="ExternalOutput")

    with tile.TileContext(nc) as tc:
        with (
            tc.tile_pool(name="const", bufs=1) as cpool,
            tc.tile_pool(name="xload", bufs=1) as xl,
            tc.tile_pool(name="x2buf", bufs=1) as xp,
            tc.tile_pool(name="csb", bufs=1) as cp,
            tc.tile_pool(name="tmp", bufs=1) as tp,
            tc.tile_pool(name="stgE", bufs=2) as se,
            tc.tile_pool(name="stgO", bufs=2) as so,
            tc.tile_pool(name="psum", bufs=1, space="PSUM") as ps,
        ):
            wt_f = cpool.tile([128, NCH * 256], f32)
            nc.sync.dma_start(wt_f[:], wt[:])
            wt_sb = cpool.tile([128, NCH * 256], bf16)
            nc.scalar.activation(wt_sb[:], wt_f[:],
                                 mybir.ActivationFunctionType.Copy)
            whv_sb = cpool.tile([128, 1], f32)
            nc.sync.dma_start(whv_sb[:], whv[:])
            ws2_sb = cpool.tile([128, PT], f32)
            nc.sync.dma_start(ws2_sb[:], ws2[:])
            vi_sb = cpool.tile([128, 1], f32)
            nc.sync.dma_start(vi_sb[:], vinit[:])

            # x2: squared input, bf16, rows = [h0-even | h0-odd | h1-even |
            # h1-odd] 32-row blocks, each holding 3 chunk groups + ones row.
            x2_sb = xp.tile([128, SCOL], bf16)
            piece = (SCOL + NLOAD - 1) // NLOAD
            for q in range(NLOAD):
                a = q * piece
                b = min(SCOL, a + piece)
                xs = xl.tile([128, b - a], f32, tag=f"x{q % 2}")
                nc.sync.dma_start(xs[:], xin[:, a:b])
                nc.scalar.activation(x2_sb[:, a:b], xs[:],
                                     mybir.ActivationFunctionType.Square)

            stage_e = None
            stage_o = None
            prev_e = None
            for s in range(NTILES):
                ch = s // TPC
                slot = s % TPC
                if slot == 0:
                    prev_e = stage_e
                    stage_e = se.tile([128, PPC], fp16, tag="se")
                    stage_o = so.tile([128, PPC], fp16, tag="so")

                m0 = slot * PT
                pu = ps.tile([128, PT], f32, tag=f"u{s % 4}")
                pc = ps.tile([128, PT], f32, tag=f"c{s % 4}")
                # block-diagonal 128-contract matmuls: both halves at once
                nc.tensor.matmul(
                    pu[:, :], wt_sb[:, 256 * ch:256 * ch + 128],
                    x2_sb[:, m0:m0 + PT],
                    start=True, stop=True, tile_position=(0, 0),
                )
                nc.tensor.matmul(
                    pc[:, :], wt_sb[:, 256 * ch + 128:256 * ch + 256],
                    x2_sb[:, m0 + 1:m0 + PT + 1],
                    start=True, stop=True, tile_position=(0, 0),
                )

                initial = (
                    vi_sb[:, 0:1] if s == 0
                    else (stage_e[:, m0 - 1:m0] if slot > 0
                          else prev_e[:, PPC - 1:PPC])
                )
                nc.vector.tensor_tensor_scan(
                    stage_e[:, m0:m0 + PT], ws2_sb[:], pu[:], initial,
                    mybir.AluOpType.mult, mybir.AluOpType.add,
                )

                if s % DVE_ODD_MOD == 0:
                    # odd path A: fused on DVE straight from PSUM
                    nc.vector.scalar_tensor_tensor(
                        stage_o[:, m0:m0 + PT], stage_e[:, m0:m0 + PT],
                        whv_sb[:, 0:1], pc[:],
                        mybir.AluOpType.mult, mybir.AluOpType.add,
                    )
                else:
                    # odd path B: ACT evacuate + ACT scale + GPSIMD add
                    csb = cp.tile([128, PT], f32, tag=f"cs{s % 4}")
                    nc.scalar.activation(
                        csb[:], pc[:], mybir.ActivationFunctionType.Copy
                    )
                    tmp = tp.tile([128, PT], fp16, tag=f"t{s % 4}")
                    nc.scalar.activation(
                        tmp[:], stage_e[:, m0:m0 + PT],
                        mybir.ActivationFunctionType.Copy,
                        scale=whv_sb[:, 0:1],
                    )
                    nc.gpsimd.tensor_tensor(
                        stage_o[:, m0:m0 + PT], tmp[:], csb[:],
                        mybir.AluOpType.add,
                    )

                if slot == TPC - 1:
                    nc.sync.dma_start(
                        voutE[:, ch * PPC:(ch + 1) * PPC], stage_e[:]
                    )
                    nc.sync.dma_start(
                        voutO[:, ch * PPC:(ch + 1) * PPC], stage_o[:]
                    )

    nc.compile()
    return nc


def _host_prep(series, vars0, bias, Wx, Wh):
    series = np.asarray(series, dtype=np.float32)
    vars0 = np.asarray(vars0, dtype=np.float64)
    bias = np.asarray(bias, dtype=np.float64)
    Wx = np.asarray(Wx, dtype=np.float64)
    Wh = np.asarray(Wh, dtype=np.float64)

    biasp = bias + 1e-6
    # wt free layout per chunk ch: [Wu-big (128 wide) | Wc-big (128 wide)],
    # both block-diagonal over the two halves (rows 64h+* -> cols 64h+*).
    wt = np.zeros((128, NCH * 256), dtype=np.float64)
    for c in range(NCH):
        for h in range(2):
            u0 = 256 * c + 64 * h
            c0 = 256 * c + 128 + 64 * h
            r = 64 * h
            for j in range(NJ):
                wt[r + 8 * c + j, u0:u0 + 64] = SCALE * Wx[:, j]
                wt[r + 32 + 8 * c + j, u0:u0 + 64] = SCALE * Wx[:, j] * Wh
                wt[r + 32 + 8 * c + j, c0:c0 + 64] = SCALE * Wx[:, j]
            wt[r + 24, u0:u0 + 64] = SCALE * biasp
            wt[r + 56, u0:u0 + 64] = SCALE * biasp * Wh
            wt[r + 56, c0:c0 + 64] = SCALE * biasp
    wt = wt.astype(np.float32)

    whv = np.tile(Wh, 2).reshape(128, 1).astype(np.float32)
    ws2 = np.repeat(
        np.tile(Wh * Wh, 2).reshape(128, 1), PT, axis=1
    ).astype(np.float32)

    in_maps = []
    for i in range(NCORES):
        xin = np.zeros((128, SCOL), dtype=np.float32)
        vinit = np.zeros((128, 1), dtype=np.float32)
        for h in range(2):
            t0 = i * 65536 + h * HALF
            first = i == 0 and h == 0
            s0 = t0 - (0 if first else W)
            for c in range(NCH):
                tc0 = s0 + c * CHUNK
                er = 64 * h + 8 * c
                orr = 64 * h + 32 + 8 * c
                # even cols j=0..PPC-1: series[tc0 + 2j]
                xin[er:er + 8, 0:PPC] = series[tc0:tc0 + CHUNK:2, :].T
                # odd cols j=0..PPC: series[tc0 + 2j - 1]
                start = tc0 - 1
                j0 = 0 if start >= 0 else (1 - start) // 2
                ov = np.zeros((SCOL, NJ), dtype=np.float32)
                ov[j0:, :] = series[start + 2 * j0:start + 2 * SCOL:2, :]
                xin[orr:orr + 8, :] = ov.T
            xin[64 * h + 24, :] = 1.0
            xin[64 * h + 56, :] = 1.0
        if i == 0:
            w0 = SCALE * (vars0 - biasp) / np.maximum(Wh, 1e-20)
            w0 = np.where(Wh < 1e-20, 0.0, w0)
            vinit[0:64, 0] = w0.astype(np.float32)
        in_maps.append({
            "xin": xin, "wt": wt, "whv": whv, "ws2": ws2, "vinit": vinit,
        })
    return in_maps


def _assemble(results):
    hist = np.empty((T, K), dtype=np.float32)
    inv = np.float32(1.0 / SCALE)
    for i in range(NCORES):
        vE = results[i]["voutE"]
        vO = results[i]["voutO"]
        for h in range(2):
            t0 = i * 65536 + h * HALF
            q0 = 0 if (i == 0 and h == 0) else W // 2
            e = vE[64 * h:64 * h + 64, q0:q0 + HALF // 2].astype(np.float32)
            o = vO[64 * h:64 * h + 64, q0:q0 + HALF // 2].astype(np.float32)
            blk = hist[t0:t0 + HALF, :]
            blk[0::2, :] = e.T * inv
            blk[1::2, :] = o.T * inv
    return hist


def run(inputs, trace=False, **kw):
    from concourse.bass_utils import run_bass_kernel_spmd

    if "nc" not in _CACHE:
        _CACHE["nc"] = _build_nc()
    nc = _CACHE["nc"]
    in_maps = _host_prep(
        inputs["series"], inputs["vars0"], inputs["bias"],
        inputs["Wx"], inputs["Wh"],
    )
    res = run_bass_kernel_spmd(
        nc, in_maps, core_ids=list(range(NCORES)), trace=trace, **kw
    )
    return _assemble(res.results), res


def kernel(series, vars0, bias, Wx, Wh):
    out, _ = run(
        {"series": series, "vars0": vars0, "bias": bias, "Wx": Wx, "Wh": Wh}
    )
    return out


# revision 17
# speedup vs baseline: 1.0601x; 1.0601x over previous
"""MixGARCH Trainium2 kernel (stride-3 decimated scan, lag-split input).

Reference: scan over t of v_t = relu(bias + Wx @ o_t^2 + Wh * v_{t-1}) + 1e-6.
All quantities >= 0 so relu is identity -> LINEAR diagonal recurrence
    v_t = Wh v_{t-1} + c_t,    c_t = (bias + 1e-6) + Wx @ o_t^2

Per core (8 cores, full I/O): 65536 steps as 2 halves (lanes 64h+k).
Non-initial halves get a 1024-step warmup; core0/half0 uses the exact v0.

Stride-3 decimation: with w_m = v_{3m},
    w_m = Wh^3 w_{m-1} + u_m,   u_m = c_{3m} + Wh c_{3m-1} + Wh^2 c_{3m-2}
    v_{3m+1} = Wh w_m + c_{3m+1},   v_{3m+2} = Wh v_{3m+1} + c_{3m+2}
The DVE scan runs at ONE-THIRD resolution. Residues are reconstructed as
out_r = Wh*out_{r-1} + c_r, balanced across engines per unit: DVE
scalar_tensor_tensor straight from PSUM, or ACT evacuate + ACT per-partition
scale + GPSIMD add (GPSIMD cannot read PSUM).

Input is packed lag-separated: for grid col m, lag-d row holds o^2(3m-d),
so u is ONE 128-contract block-diagonal matmul (both halves at once; no PSUM
accumulation groups, which break codegen here), rhs contiguous. The residue
c-streams read the lag-(3-r) rows at col m+1. 2 chunks/half overlay on
distinct row groups; per-chunk weight variants select rows. Bias rides a
ones-channel row (summed geometric coefficients for u).

The linear system is scaled x1024 so outputs fit fp16 normal range (halves
output DMA bytes). Host interleaves the 3 streams and divides by 1024.
"""

import os
import numpy as np

T = 524288
K = 64
NJ = 8
NCORES = 8
W = 1024                # warmup steps per half
HALF = 32768            # real steps per half
TT = W + HALF           # 33792 steps per half-stream
NCH = 2                 # chunks per half
CHUNK = TT // NCH       # 16896 timesteps per chunk
GRID = TT // 3          # 11264 grid points per half
GPC = CHUNK // 3        # 5632 grid points per chunk
SCOL = GPC + 1          # stored cols per lag row
PT = 512                # grid points per tile
TPC = GPC // PT         # 11 tiles per chunk
NTILES = NCH * TPC      # 22
SCALE = 1024.0
NLOAD = 4
ODD_MOD = int(os.environ.get("MIXGARCH_DVE_ODD_MOD", "3"))

_CACHE = {}


def _build_nc():
    import concourse.bacc as bacc
    import concourse.mybir as mybir
    import concourse.tile as tile

    f32 = mybir.dt.float32
    bf16 = mybir.dt.bfloat16
    fp16 = mybir.dt.float16

    nc = bacc.Bacc(None, target_bir_lowering=False)
    xin = nc.dram_tensor("xin", [128, SCOL], f32, kind="ExternalInput")
    wt = nc.dram_tensor("wt", [128, NCH * 384], f32, kind="ExternalInput")
    whv = nc.dram_tensor("whv", [128, 1], f32, kind="ExternalInput")
    ws3 = nc.dram_tensor("ws3", [128, PT], f32, kind="ExternalInput")
    vinit = nc.dram_tensor("vinit", [128, 1], f32, kind="ExternalInput")
    vo = [
        nc.dram_tensor(f"vout{r}", [128, GRID], fp16, kind="ExternalOutput")
        for r in range(3)
    ]

    with tile.TileContext(nc) as tc:
        with (
            tc.tile_pool(name="const", bufs=1) as cpool,
            tc.tile_pool(name="xload", bufs=1) as xl,
            tc.tile_pool(name="x2buf", bufs=1) as xp,
            tc.tile_pool(name="csb", bufs=1) as cp,
            tc.tile_pool(name="tmp", bufs=1) as tp,
            tc.tile_pool(name="stg0", bufs=2) as s0p,
            tc.tile_pool(name="stg1", bufs=2) as s1p,
            tc.tile_pool(name="stg2", bufs=2) as s2p,
            tc.tile_pool(name="psum", bufs=1, space="PSUM") as ps,
        ):
            wt_f = cpool.tile([128, NCH * 384], f32)
            nc.sync.dma_start(wt_f[:], wt[:])
            wt_sb = cpool.tile([128, NCH * 384], bf16)
            nc.scalar.activation(wt_sb[:], wt_f[:],
                                 mybir.ActivationFunctionType.Copy)
            whv_sb = cpool.tile([128, 1], f32)
            nc.sync.dma_start(whv_sb[:], whv[:])
            ws3_sb = cpool.tile([128, PT], f32)
            nc.sync.dma_start(ws3_sb[:], ws3[:])
            vi_sb = cpool.tile([128, 1], f32)
            nc.sync.dma_start(vi_sb[:], vinit[:])

            x2_sb = xp.tile([128, SCOL], bf16)
            piece = (SCOL + NLOAD - 1) // NLOAD
            for q in range(NLOAD):
                a = q * piece
                b = min(SCOL, a + piece)
                xs = xl.tile([128, b - a], f32, tag=f"x{q % 2}")
                nc.sync.dma_start(xs[:], xin[:, a:b])
                nc.scalar.activation(x2_sb[:, a:b], xs[:],
                                     mybir.ActivationFunctionType.Square)

            stg = [None, None, None]
            prev0 = None
            for s in range(NTILES):
                ch = s // TPC
                slot = s % TPC
                if slot == 0:
                    prev0 = stg[0]
                    st0 = s0p.tile([128, GPC], fp16, tag="s0", name="st0")
                    st1 = s1p.tile([128, GPC], fp16, tag="s1", name="st1")
                    st2 = s2p.tile([128, GPC], fp16, tag="s2", name="st2")
                    stg = [st0, st1, st2]

                m0 = slot * PT
                w0 = 384 * ch
                pu = ps.tile([128, PT], f32, tag=f"u{s % 2}")
                pc1 = ps.tile([128, PT], f32, tag=f"c1{s % 2}")
                pc2 = ps.tile([128, PT], f32, tag=f"c2{s % 2}")
                nc.tensor.matmul(
                    pu[:, :], wt_sb[:, w0:w0 + 128],
                    x2_sb[:, m0:m0 + PT],
                    start=True, stop=True, tile_position=(0, 0),
                )
                nc.tensor.matmul(
                    pc1[:, :], wt_sb[:, w0 + 128:w0 + 256],
                    x2_sb[:, m0 + 1:m0 + PT + 1],
                    start=True, stop=True, tile_position=(0, 0),
                )
                nc.tensor.matmul(
                    pc2[:, :], wt_sb[:, w0 + 256:w0 + 384],
                    x2_sb[:, m0 + 1:m0 + PT + 1],
                    start=True, stop=True, tile_position=(0, 0),
                )

                initial = (
                    vi_sb[:, 0:1] if s == 0
                    else (stg[0][:, m0 - 1:m0] if slot > 0
                          else prev0[:, GPC - 1:GPC])
                )
                nc.vector.tensor_tensor_scan(
                    stg[0][:, m0:m0 + PT], ws3_sb[:], pu[:], initial,
                    mybir.AluOpType.mult, mybir.AluOpType.add,
                )

                for r in (1, 2):
                    pc = pc1 if r == 1 else pc2
                    src = stg[r - 1][:, m0:m0 + PT]
                    dst = stg[r][:, m0:m0 + PT]
                    if (2 * s + r - 1) % ODD_MOD == 0:
                        nc.vector.scalar_tensor_tensor(
                            dst, src, whv_sb[:, 0:1], pc[:],
                            mybir.AluOpType.mult, mybir.AluOpType.add,
                        )
                    else:
                        csb = cp.tile([128, PT], f32, tag=f"cs{(2*s+r) % 4}")
                        nc.scalar.activation(
                            csb[:], pc[:], mybir.ActivationFunctionType.Copy
                        )
                        tmp = tp.tile([128, PT], fp16, tag=f"t{(2*s+r) % 4}")
                        nc.scalar.activation(
                            tmp[:], src, mybir.ActivationFunctionType.Copy,
                            scale=whv_sb[:, 0:1],
                        )
                        nc.gpsimd.tensor_tensor(
                            dst, tmp[:], csb[:], mybir.AluOpType.add,
                        )

                if slot == TPC - 1:
                    for r in range(3):
                        nc.sync.dma_start(
                            vo[r][:, ch * GPC:(ch + 1) * GPC], stg[r][:]
                        )

    nc.compile()
    return nc


def _host_prep(series, vars0, bias, Wx, Wh):
    series = np.asarray(series, dtype=np.float32)
    vars0 = np.asarray(vars0, dtype=np.float64)
    bias = np.asarray(bias, dtype=np.float64)
    Wx = np.asarray(Wx, dtype=np.float64)
    Wh = np.asarray(Wh, dtype=np.float64)

    biasp = bias + 1e-6
    # wt free layout per chunk ch: [Wu | Wc1 | Wc2], each 128 wide,
    # block-diagonal over halves (rows 64h+* -> cols 64h+*).
    # Row groups per half: lag-d of chunk c at 64h + 8*(3c+d); ones at 64h+48.
    # Wu: lag-d rows get S*Wh^d*Wx; ones = S*biasp*(1+Wh+Wh^2).
    # Wc1 (c at 3m+1 = lag-2 col m+1): lag-2 rows get S*Wx; ones = S*biasp.
    # Wc2 (c at 3m+2 = lag-1 col m+1): lag-1 rows get S*Wx; ones = S*biasp.
    wt = np.zeros((128, NCH * 384), dtype=np.float64)
    whp = [np.ones_like(Wh), Wh, Wh * Wh]
    for c in range(NCH):
        for h in range(2):
            u0 = 384 * c + 64 * h
            c1 = 384 * c + 128 + 64 * h
            c2 = 384 * c + 256 + 64 * h
            rr = 64 * h
            for d in range(3):
                base = rr + 8 * (3 * c + d)
                for j in range(NJ):
                    wt[base + j, u0:u0 + 64] = SCALE * Wx[:, j] * whp[d]
            for j in range(NJ):
                wt[rr + 8 * (3 * c + 2) + j, c1:c1 + 64] = SCALE * Wx[:, j]
                wt[rr + 8 * (3 * c + 1) + j, c2:c2 + 64] = SCALE * Wx[:, j]
            wt[rr + 48, u0:u0 + 64] = SCALE * biasp * (1 + Wh + Wh * Wh)
            wt[rr + 48, c1:c1 + 64] = SCALE * biasp
            wt[rr + 48, c2:c2 + 64] = SCALE * biasp
    wt = wt.astype(np.float32)

    whv = np.tile(Wh, 2).reshape(128, 1).astype(np.float32)
    ws3 = np.repeat(
        np.tile(Wh ** 3, 2).reshape(128, 1), PT, axis=1
    ).astype(np.float32)

    in_maps = []
    for i in range(NCORES):
        xin = np.zeros((128, SCOL), dtype=np.float32)
        vinit = np.zeros((128, 1), dtype=np.float32)
        for h in range(2):
            t0 = i * 65536 + h * HALF
            first = i == 0 and h == 0
            s0 = t0 - (0 if first else W)
            for c in range(NCH):
                tc0 = s0 + c * CHUNK
                for d in range(3):
                    row = 64 * h + 8 * (3 * c + d)
                    start = tc0 - d
                    j0 = 0 if start >= 0 else (2 - start) // 3
                    ov = np.zeros((SCOL, NJ), dtype=np.float32)
                    seq = series[start + 3 * j0:start + 3 * SCOL:3, :]
                    ov[j0:j0 + seq.shape[0], :] = seq
                    xin[row:row + 8, :] = ov.T
            xin[64 * h + 48, :] = 1.0
        if i == 0:
            w0v = SCALE * (vars0 - (1 + Wh) * biasp) / np.maximum(Wh, 1e-11) ** 2
            w0v = np.where(Wh < 1e-11, 0.0, w0v)
            vinit[0:64, 0] = w0v.astype(np.float32)
        in_maps.append({
            "xin": xin, "wt": wt, "whv": whv, "ws3": ws3, "vinit": vinit,
        })
    return in_maps


def _assemble(results):
    hist = np.empty((T, K), dtype=np.float32)
    inv = np.float32(1.0 / SCALE)
    F = np.empty((TT, K), dtype=np.float32)
    for i in range(NCORES):
        for h in range(2):
            t0 = i * 65536 + h * HALF
            w0 = 0 if (i == 0 and h == 0) else W
            for r in range(3):
                F[r::3, :] = (
                    results[i][f"vout{r}"][64 * h:64 * h + 64, :]
                    .astype(np.float32).T
                )
            hist[t0:t0 + HALF, :] = F[w0:w0 + HALF, :] * inv
    return hist


def run(inputs, trace=False, **kw):
    from concourse.bass_utils import run_bass_kernel_spmd

    if "nc" not in _CACHE:
        _CACHE["nc"] = _build_nc()
    nc = _CACHE["nc"]
    in_maps = _host_prep(
        inputs["series"], inputs["vars0"], inputs["bias"],
        inputs["Wx"], inputs["Wh"],
    )
    res = run_bass_kernel_spmd(
        nc, in_maps, core_ids=list(range(NCORES)), trace=trace, **kw
    )
    return _assemble(res.results), res


def kernel(series, vars0, bias, Wx, Wh):
    out, _ = run(
        {"series": series, "vars0": vars0, "bias": bias, "Wx": Wx, "Wh": Wh}
    )
    return out


# revision 18
# speedup vs baseline: 1.1901x; 1.1227x over previous
"""MixGARCH Trainium2 kernel (stride-3 decimated scan, lag-split input).

Reference: scan over t of v_t = relu(bias + Wx @ o_t^2 + Wh * v_{t-1}) + 1e-6.
All quantities >= 0 so relu is identity -> LINEAR diagonal recurrence
    v_t = Wh v_{t-1} + c_t,    c_t = (bias + 1e-6) + Wx @ o_t^2

Per core (8 cores, full I/O): 65536 steps as 2 halves (lanes 64h+k).
Non-initial halves get a 1024-step warmup; core0/half0 uses the exact v0.

Stride-3 decimation: with w_m = v_{3m},
    w_m = Wh^3 w_{m-1} + u_m,   u_m = c_{3m} + Wh c_{3m-1} + Wh^2 c_{3m-2}
    v_{3m+1} = Wh w_m + c_{3m+1},   v_{3m+2} = Wh v_{3m+1} + c_{3m+2}
The DVE scan runs at ONE-THIRD resolution. Residues are reconstructed as
out_r = Wh*out_{r-1} + c_r, balanced across engines per unit: DVE
scalar_tensor_tensor straight from PSUM, or ACT evacuate + ACT per-partition
scale + GPSIMD add (GPSIMD cannot read PSUM).

Input is packed lag-separated: for grid col m, lag-d row holds o^2(3m-d),
so u is ONE 128-contract block-diagonal matmul (both halves at once; no PSUM
accumulation groups, which break codegen here), rhs contiguous. The residue
c-streams read the lag-(3-r) rows at col m+1. 2 chunks/half overlay on
distinct row groups; per-chunk weight variants select rows. Bias rides a
ones-channel row (summed geometric coefficients for u).

The linear system is scaled x1024 so outputs fit fp16 normal range (halves
output DMA bytes). Host interleaves the 3 streams and divides by 1024.
"""

import os
import numpy as np

T = 524288
K = 64
NJ = 8
NCORES = 8
W = 1024                # warmup steps per half
HALF = 32768            # real steps per half
TT = W + HALF           # 33792 steps per half-stream
NCH = 2                 # chunks per half
CHUNK = TT // NCH       # 16896 timesteps per chunk
GRID = TT // 3          # 11264 grid points per half
GPC = CHUNK // 3        # 5632 grid points per chunk
SCOL = GPC + 1          # stored cols per lag row
PT = 512                # grid points per tile
TPC = GPC // PT         # 11 tiles per chunk
NTILES = NCH * TPC      # 22
SCALE = 1024.0
NLOAD = 8
ODD_MOD = int(os.environ.get("MIXGARCH_DVE_ODD_MOD", "3"))

_CACHE = {}


def _build_nc():
    import concourse.bacc as bacc
    import concourse.mybir as mybir
    import concourse.tile as tile

    f32 = mybir.dt.float32
    bf16 = mybir.dt.bfloat16
    fp16 = mybir.dt.float16

    nc = bacc.Bacc(None, target_bir_lowering=False)
    xin = nc.dram_tensor("xin", [128, SCOL], f32, kind="ExternalInput")
    wt = nc.dram_tensor("wt", [128, NCH * 384], f32, kind="ExternalInput")
    whv = nc.dram_tensor("whv", [128, 1], f32, kind="ExternalInput")
    ws3 = nc.dram_tensor("ws3", [128, PT], f32, kind="ExternalInput")
    vinit = nc.dram_tensor("vinit", [128, 1], f32, kind="ExternalInput")
    vo = [
        nc.dram_tensor(f"vout{r}", [128, GRID], fp16, kind="ExternalOutput")
        for r in range(3)
    ]

    with tile.TileContext(nc) as tc:
        with (
            tc.tile_pool(name="const", bufs=1) as cpool,
            tc.tile_pool(name="xload", bufs=1) as xl,
            tc.tile_pool(name="x2buf", bufs=1) as xp,
            tc.tile_pool(name="csb", bufs=1) as cp,
            tc.tile_pool(name="tmp", bufs=1) as tp,
            tc.tile_pool(name="stg0", bufs=2) as s0p,
            tc.tile_pool(name="stg1", bufs=2) as s1p,
            tc.tile_pool(name="stg2", bufs=2) as s2p,
            tc.tile_pool(name="psum", bufs=1, space="PSUM") as ps,
        ):
            wt_f = cpool.tile([128, NCH * 384], f32)
            nc.sync.dma_start(wt_f[:], wt[:])
            wt_sb = cpool.tile([128, NCH * 384], bf16)
            nc.scalar.activation(wt_sb[:], wt_f[:],
                                 mybir.ActivationFunctionType.Copy)
            whv_sb = cpool.tile([128, 1], f32)
            nc.sync.dma_start(whv_sb[:], whv[:])
            ws3_sb = cpool.tile([128, PT], f32)
            nc.sync.dma_start(ws3_sb[:], ws3[:])
            vi_sb = cpool.tile([128, 1], f32)
            nc.sync.dma_start(vi_sb[:], vinit[:])

            x2_sb = xp.tile([128, SCOL], bf16)
            piece = (SCOL + NLOAD - 1) // NLOAD
            for q in range(NLOAD):
                a = q * piece
                b = min(SCOL, a + piece)
                xs = xl.tile([128, b - a], f32, tag=f"x{q % 2}")
                nc.sync.dma_start(xs[:], xin[:, a:b])
                nc.scalar.activation(x2_sb[:, a:b], xs[:],
                                     mybir.ActivationFunctionType.Square)

            stg = [None, None, None]
            prev0 = None
            for s in range(NTILES):
                ch = s // TPC
                slot = s % TPC
                if slot == 0:
                    prev0 = stg[0]
                    st0 = s0p.tile([128, GPC], fp16, tag="s0", name="st0")
                    st1 = s1p.tile([128, GPC], fp16, tag="s1", name="st1")
                    st2 = s2p.tile([128, GPC], fp16, tag="s2", name="st2")
                    stg = [st0, st1, st2]

                m0 = slot * PT
                w0 = 384 * ch
                pu = ps.tile([128, PT], f32, tag=f"u{s % 2}")
                pc1 = ps.tile([128, PT], f32, tag=f"c1{s % 2}")
                pc2 = ps.tile([128, PT], f32, tag=f"c2{s % 2}")
                nc.tensor.matmul(
                    pu[:, :], wt_sb[:, w0:w0 + 128],
                    x2_sb[:, m0:m0 + PT],
                    start=True, stop=True, tile_position=(0, 0),
                )
                nc.tensor.matmul(
                    pc1[:, :], wt_sb[:, w0 + 128:w0 + 256],
                    x2_sb[:, m0 + 1:m0 + PT + 1],
                    start=True, stop=True, tile_position=(0, 0),
                )
                nc.tensor.matmul(
                    pc2[:, :], wt_sb[:, w0 + 256:w0 + 384],
                    x2_sb[:, m0 + 1:m0 + PT + 1],
                    start=True, stop=True, tile_position=(0, 0),
                )

                initial = (
                    vi_sb[:, 0:1] if s == 0
                    else (stg[0][:, m0 - 1:m0] if slot > 0
                          else prev0[:, GPC - 1:GPC])
                )
                nc.vector.tensor_tensor_scan(
                    stg[0][:, m0:m0 + PT], ws3_sb[:], pu[:], initial,
                    mybir.AluOpType.mult, mybir.AluOpType.add,
                )

                for r in (1, 2):
                    pc = pc1 if r == 1 else pc2
                    src = stg[r - 1][:, m0:m0 + PT]
                    dst = stg[r][:, m0:m0 + PT]
                    if (2 * s + r - 1) % ODD_MOD == 0:
                        nc.vector.scalar_tensor_tensor(
                            dst, src, whv_sb[:, 0:1], pc[:],
                            mybir.AluOpType.mult, mybir.AluOpType.add,
                        )
                    else:
                        csb = cp.tile([128, PT], f32, tag=f"cs{(2*s+r) % 4}")
                        nc.scalar.activation(
                            csb[:], pc[:], mybir.ActivationFunctionType.Copy
                        )
                        tmp = tp.tile([128, PT], fp16, tag=f"t{(2*s+r) % 4}")
                        nc.scalar.activation(
                            tmp[:], src, mybir.ActivationFunctionType.Copy,
                            scale=whv_sb[:, 0:1],
                        )
                        nc.gpsimd.tensor_tensor(
                            dst, tmp[:], csb[:], mybir.AluOpType.add,
                        )

                if slot % 2 == 1 or slot == TPC - 1:
                    lo = (slot // 2) * 2 * PT if slot % 2 == 1 else (slot // 2) * 2 * PT
                    hi = m0 + PT
                    for r in range(3):
                        nc.sync.dma_start(
                            vo[r][:, ch * GPC + lo:ch * GPC + hi],
                            stg[r][:, lo:hi],
                        )

    nc.compile()
    return nc


def _host_prep(series, vars0, bias, Wx, Wh):
    series = np.asarray(series, dtype=np.float32)
    vars0 = np.asarray(vars0, dtype=np.float64)
    bias = np.asarray(bias, dtype=np.float64)
    Wx = np.asarray(Wx, dtype=np.float64)
    Wh = np.asarray(Wh, dtype=np.float64)

    biasp = bias + 1e-6
    # wt free layout per chunk ch: [Wu | Wc1 | Wc2], each 128 wide,
    # block-diagonal over halves (rows 64h+* -> cols 64h+*).
    # Row groups per half: lag-d of chunk c at 64h + 8*(3c+d); ones at 64h+48.
    # Wu: lag-d rows get S*Wh^d*Wx; ones = S*biasp*(1+Wh+Wh^2).
    # Wc1 (c at 3m+1 = lag-2 col m+1): lag-2 rows get S*Wx; ones = S*biasp.
    # Wc2 (c at 3m+2 = lag-1 col m+1): lag-1 rows get S*Wx; ones = S*biasp.
    wt = np.zeros((128, NCH * 384), dtype=np.float64)
    whp = [np.ones_like(Wh), Wh, Wh * Wh]
    for c in range(NCH):
        for h in range(2):
            u0 = 384 * c + 64 * h
            c1 = 384 * c + 128 + 64 * h
            c2 = 384 * c + 256 + 64 * h
            rr = 64 * h
            for d in range(3):
                base = rr + 8 * (3 * c + d)
                for j in range(NJ):
                    wt[base + j, u0:u0 + 64] = SCALE * Wx[:, j] * whp[d]
            for j in range(NJ):
                wt[rr + 8 * (3 * c + 2) + j, c1:c1 + 64] = SCALE * Wx[:, j]
                wt[rr + 8 * (3 * c + 1) + j, c2:c2 + 64] = SCALE * Wx[:, j]
            wt[rr + 48, u0:u0 + 64] = SCALE * biasp * (1 + Wh + Wh * Wh)
            wt[rr + 48, c1:c1 + 64] = SCALE * biasp
            wt[rr + 48, c2:c2 + 64] = SCALE * biasp
    wt = wt.astype(np.float32)

    whv = np.tile(Wh, 2).reshape(128, 1).astype(np.float32)
    ws3 = np.repeat(
        np.tile(Wh ** 3, 2).reshape(128, 1), PT, axis=1
    ).astype(np.float32)

    in_maps = []
    for i in range(NCORES):
        xin = np.zeros((128, SCOL), dtype=np.float32)
        vinit = np.zeros((128, 1), dtype=np.float32)
        for h in range(2):
            t0 = i * 65536 + h * HALF
            first = i == 0 and h == 0
            s0 = t0 - (0 if first else W)
            for c in range(NCH):
                tc0 = s0 + c * CHUNK
                for d in range(3):
                    row = 64 * h + 8 * (3 * c + d)
                    start = tc0 - d
                    j0 = 0 if start >= 0 else (2 - start) // 3
                    ov = np.zeros((SCOL, NJ), dtype=np.float32)
                    seq = series[start + 3 * j0:start + 3 * SCOL:3, :]
                    ov[j0:j0 + seq.shape[0], :] = seq
                    xin[row:row + 8, :] = ov.T
            xin[64 * h + 48, :] = 1.0
        if i == 0:
            w0v = SCALE * (vars0 - (1 + Wh) * biasp) / np.maximum(Wh, 1e-11) ** 2
            w0v = np.where(Wh < 1e-11, 0.0, w0v)
            vinit[0:64, 0] = w0v.astype(np.float32)
        in_maps.append({
            "xin": xin, "wt": wt, "whv": whv, "ws3": ws3, "vinit": vinit,
        })
    return in_maps


def _assemble(results):
    hist = np.empty((T, K), dtype=np.float32)
    inv = np.float32(1.0 / SCALE)
    F = np.empty((TT, K), dtype=np.float32)
    for i in range(NCORES):
        for h in range(2):
            t0 = i * 65536 + h * HALF
            w0 = 0 if (i == 0 and h == 0) else W
            for r in range(3):
                F[r::3, :] = (
                    results[i][f"vout{r}"][64 * h:64 * h + 64, :]
                    .astype(np.float32).T
                )
            hist[t0:t0 + HALF, :] = F[w0:w0 + HALF, :] * inv
    return hist


def run(inputs, trace=False, **kw):
    from concourse.bass_utils import run_bass_kernel_spmd

    if "nc" not in _CACHE:
        _CACHE["nc"] = _build_nc()
    nc = _CACHE["nc"]
    in_maps = _host_prep(
        inputs["series"], inputs["vars0"], inputs["bias"],
        inputs["Wx"], inputs["Wh"],
    )
    res = run_bass_kernel_spmd(
        nc, in_maps, core_ids=list(range(NCORES)), trace=trace, **kw
    )
    return _assemble(res.results), res


def kernel(series, vars0, bias, Wx, Wh):
    out, _ = run(
        {"series": series, "vars0": vars0, "bias": bias, "Wx": Wx, "Wh": Wh}
    )
    return out
